# revision 3
# baseline (speedup 1.0000x reference)
"""Multi-head attention (B=4, S=2048, E=1024, H=16, D=64) on 8 TRN2 NeuronCores.

Sharding: core c handles batch b = c//2 and heads [8*(c%2), 8*(c%2)+8) —
data parallel over batch, tensor parallel over heads. No collectives:
each core computes its own output slice, gathered on host.

Per-core algorithm (all matmuls in float32r = full-rate fp32):
  qT = (Wq_slice)^T-free matmul:  qT[f, s]  = sum_e Wq[e, f] * XqT[e, s]
  kT likewise; v[s, f] = sum_e XvT[e, s] * Wv[e, f]  (natural layout)
  per head h, sq-chunk j (512 wide):
    S^T[sk_blk, sq] = matmul(lhsT=kT_h[:, blk], rhs=qT_h[:, j])   (K=64)
    P = exp(S^T / 8)            (ACT, batched over 2 psum banks)
    ctx^T[0:64, sq] += matmul(lhsT=[v_h | 1], rhs=P)  -> row 64 = sum(P)
  output per core: [8 heads, 65, 2048]; host divides rows 0..63 by row 64,
  transposes, and scatters into the full [4, 2048, 1024] result.
"""

import numpy as np
from contextlib import ExitStack

import concourse.bass as bass
import concourse.tile as tile
from concourse import bacc
from concourse import mybir
from concourse.bass_utils import run_bass_kernel_spmd

F32 = mybir.dt.float32
F32R = mybir.dt.float32r
EXP = mybir.ActivationFunctionType.Exp

B, S, E = 4, 2048, 1024
H, D = 16, 64
HPC = 8            # heads per core
FPC = HPC * D      # 512 output features per core
N_CORES = 8
KC = E // 128      # contraction chunks
NJ = S // 512      # sq chunks
NT = S // 128      # sk blocks
SCALE = 0.125      # 1/sqrt(64)


def build_bass(repeat=1):
    nc = bacc.Bacc()
    xq = nc.declare_dram_parameter("xq_t", [E, S], F32R, isOutput=False)
    xk = nc.declare_dram_parameter("xk_t", [E, S], F32R, isOutput=False)
    xv = nc.declare_dram_parameter("xv_t", [E, S], F32R, isOutput=False)
    wq = nc.declare_dram_parameter("wq", [E, FPC], F32R, isOutput=False)
    wk = nc.declare_dram_parameter("wk", [E, FPC], F32R, isOutput=False)
    wv = nc.declare_dram_parameter("wv", [E, FPC], F32R, isOutput=False)
    out = nc.declare_dram_parameter("out", [HPC, D + 1, S], F32, isOutput=True)

    with tile.TileContext(nc) as tc, ExitStack() as ctx:
        sb = ctx.enter_context(tc.tile_pool(name="sb", bufs=1))
        xs = ctx.enter_context(tc.tile_pool(name="xs", bufs=2))
        exp = ctx.enter_context(tc.tile_pool(name="exp", bufs=2))
        ps = ctx.enter_context(tc.tile_pool(name="ps", bufs=2, space="PSUM"))

        # --- weights, resident ---
        w_sb = {}
        for name, w in (("wq", wq), ("wk", wk), ("wv", wv)):
            t = sb.tile([128, KC, FPC], F32R, name=f"{name}_sb", tag=f"{name}_sb")
            nc.sync.dma_start(out=t, in_=w.rearrange("(kc p) f -> p kc f", p=128))
            w_sb[name] = t

        # --- persistent projection outputs ---
        qT = sb.tile([128, NJ, S], F32R, name="qT", tag="qT")     # [f%128, f//128, s]
        kT = sb.tile([128, NJ, S], F32R, name="kT", tag="kT")
        vaug = sb.tile([128, HPC, NT, D + 1], F32R, name="vaug", tag="vaug")
        for _h in range(HPC):
            for _t in range(NT):
                nc.vector.memset(vaug[:, _h, _t, D:D + 1].bitcast(F32), 1.0)

        for r in range(repeat):
            emit_body(nc, tc, xq, xk, xv, out, w_sb, qT, kT, vaug, xs, exp, ps, r)

    nc.compile()
    nc.freeze()
    return nc


def emit_body(nc, tc, xq, xk, xv, out, w_sb, qT, kT, vaug, xs, exp, ps, r):
    # --- q^T / k^T projections ---
    for name, x, dst in (("wq", xq, qT), ("wk", xk, kT)):
        for j in range(NJ):
            xt = xs.tile([128, KC, 512], F32R, name=f"x_{name}_{j}_{r}", tag="xt")
            nc.sync.dma_start(
                out=xt,
                in_=x[:, j * 512:(j + 1) * 512].rearrange(
                    "(kc p) f -> p kc f", p=128),
            )
            for m in range(4):  # output-feature chunks of 128
                acc = ps.tile([128, 512], F32, name=f"p_{name}_{j}_{m}_{r}",
                              tag="proj")
                for kc in range(KC):
                    nc.tensor.matmul(
                        acc,
                        lhsT=w_sb[name][:, kc, m * 128:(m + 1) * 128],
                        rhs=xt[:, kc, :],
                        start=(kc == 0), stop=(kc == KC - 1),
                    )
                nc.vector.tensor_copy(
                    out=dst[:, m, j * 512:(j + 1) * 512], in_=acc)

    # --- v projection (natural [s, f] layout) into v_aug ---
    for j in range(NJ):
        xt = xs.tile([128, KC, 512], F32R, name=f"x_v_{j}_{r}", tag="xt")
        nc.sync.dma_start(
            out=xt,
            in_=xv[:, j * 512:(j + 1) * 512].rearrange(
                "(kc p) f -> p kc f", p=128),
        )
        for sc in range(4):  # s chunks of 128 inside this j
            t = j * 4 + sc
            acc = ps.tile([128, FPC], F32, name=f"p_v_{j}_{sc}_{r}", tag="proj")
            for kc in range(KC):
                nc.tensor.matmul(
                    acc,
                    lhsT=xt[:, kc, sc * 128:(sc + 1) * 128],
                    rhs=w_sb["wv"][:, kc, :],
                    start=(kc == 0), stop=(kc == KC - 1),
                )
            for h in range(HPC):
                nc.vector.tensor_copy(
                    out=vaug[:, h, t, 0:D], in_=acc[:, h * D:(h + 1) * D])

    # --- attention ---
    for h in range(HPC):
        po = (h % 2) * 64   # partition offset of head h inside its chunk
        m = h // 2
        for j in range(NJ):
            cacc = ps.tile([D + 1, 512], F32, name=f"ctx_{h}_{j}_{r}", tag="ctx")
            for tg in range(NT // 2):   # exp over 2 banks at a time
                st = ps.tile([128, 2, 512], F32, name=f"st_{h}_{j}_{tg}_{r}",
                             tag="st")
                for u in range(2):
                    t = tg * 2 + u
                    nc.tensor.matmul(
                        st[:, u, :],
                        lhsT=kT[po:po + 64, m, t * 128:(t + 1) * 128],
                        rhs=qT[po:po + 64, m, j * 512:(j + 1) * 512],
                        start=True, stop=True,
                    )
                ex = exp.tile([128, 2, 512], F32R, name=f"ex_{h}_{j}_{tg}_{r}",
                              tag="ex")
                nc.scalar.activation(ex, st, EXP, scale=SCALE)
                for u in range(2):
                    t = tg * 2 + u
                    nc.tensor.matmul(
                        cacc,
                        lhsT=vaug[:, h, t, :],
                        rhs=ex[:, u, :],
                        start=(t == 0), stop=(t == NT - 1),
                    )
            csb = exp.tile([D + 1, 512], F32, name=f"csb_{h}_{j}_{r}",
                           tag="csb")
            nc.vector.tensor_copy(out=csb, in_=cacc)
            nc.sync.dma_start(
                out=out[h, :, j * 512:(j + 1) * 512], in_=csb)


_NC_CACHE = None


def _get_nc():
    global _NC_CACHE
    if _NC_CACHE is None:
        _NC_CACHE = build_bass()
    return _NC_CACHE


def make_in_maps(queries, keys, values, Wq, Wk, Wv):
    # Host-side shard prep: transpose activations once per batch, slice W by head.
    xq_t = [np.ascontiguousarray(queries[b].T) for b in range(B)]
    xk_t = [np.ascontiguousarray(keys[b].T) for b in range(B)]
    xv_t = [np.ascontiguousarray(values[b].T) for b in range(B)]
    w_half = [
        (np.ascontiguousarray(Wq[:, g * FPC:(g + 1) * FPC]),
         np.ascontiguousarray(Wk[:, g * FPC:(g + 1) * FPC]),
         np.ascontiguousarray(Wv[:, g * FPC:(g + 1) * FPC]))
        for g in range(2)
    ]

    in_maps = []
    for c in range(N_CORES):
        b, g = c // 2, c % 2
        in_maps.append({
            "xq_t": xq_t[b], "xk_t": xk_t[b], "xv_t": xv_t[b],
            "wq": w_half[g][0], "wk": w_half[g][1], "wv": w_half[g][2],
        })
    return in_maps


def kernel(queries, keys, values, Wq, Wk, Wv, **_):
    queries = np.asarray(queries, dtype=np.float32)
    keys = np.asarray(keys, dtype=np.float32)
    values = np.asarray(values, dtype=np.float32)
    Wq = np.asarray(Wq, dtype=np.float32)
    Wk = np.asarray(Wk, dtype=np.float32)
    Wv = np.asarray(Wv, dtype=np.float32)

    in_maps = make_in_maps(queries, keys, values, Wq, Wk, Wv)

    nc = _get_nc()
    res = run_bass_kernel_spmd(nc, in_maps, list(range(N_CORES)))

    full = np.empty((B, S, H * D), dtype=np.float32)
    for c in range(N_CORES):
        b, g = c // 2, c % 2
        o = res.results[c]["out"]          # [HPC, D+1, S]
        ctx = o[:, :D, :] / o[:, D:D + 1, :]     # [HPC, D, S]
        dst = full[b].reshape(S, H, D)
        dst[:, g * HPC:(g + 1) * HPC, :] = ctx.transpose(2, 0, 1)
    return full



# revision 4
# speedup vs baseline: 1.1928x; 1.1928x over previous
"""Multi-head attention (B=4, S=2048, E=1024, H=16, D=64) on 8 TRN2 NeuronCores.

Sharding: core c handles batch b = c//2 and heads [8*(c%2), 8*(c%2)+8) —
data parallel over batch, tensor parallel over heads. No collectives.

v5: 512-wide moving operands (ISA limit s3d3_mm_num_elements), bf16
operands (half the HBM traffic / SBUF / host transfer; PSUM accumulation
fp32), PSUM: st pairs triple-buffered (6 banks) + one shared 2-slot acc
tag (cacc + projection accumulators), ACT over 2-bank pairs (256
activations), emission order = schedule priority: j0 projections first,
head 0's t-loop split 4 ways so K/V projections of later j-chunks are
emitted exactly where needed, Q projections of chunk j+1 spread between
attention head pairs as PE filler, per-2-head batched output DMAs.
"""

import numpy as np
from contextlib import ExitStack

import concourse.bass as bass
import concourse.tile as tile
from concourse import bacc
from concourse import mybir
from concourse.bass_utils import run_bass_kernel_spmd

F32 = mybir.dt.float32
BF16 = mybir.dt.bfloat16
EXP = mybir.ActivationFunctionType.Exp

B, S, E = 4, 2048, 1024
H, D = 16, 64
HPC = 8            # heads per core
FPC = HPC * D      # 512 output features per core
N_CORES = 8
KC = E // 128      # contraction chunks
J = 512            # s chunk width
NJ = S // J        # s chunks (4)
NT = S // 128      # sk blocks (16)
SCALE = 0.125      # 1/sqrt(64)


def build_bass(repeat=1):
    nc = bacc.Bacc()
    xq = nc.declare_dram_parameter("xq_t", [E, S], BF16, isOutput=False)
    xk = nc.declare_dram_parameter("xk_t", [E, S], BF16, isOutput=False)
    xv = nc.declare_dram_parameter("xv_t", [E, S], BF16, isOutput=False)
    wq = nc.declare_dram_parameter("wq", [E, FPC], BF16, isOutput=False)
    wk = nc.declare_dram_parameter("wk", [E, FPC], BF16, isOutput=False)
    wv = nc.declare_dram_parameter("wv", [E, FPC], BF16, isOutput=False)
    out = nc.declare_dram_parameter("out", [HPC, D + 1, S], F32, isOutput=True)

    with tile.TileContext(nc) as tc, ExitStack() as ctx:
        sb = ctx.enter_context(tc.tile_pool(name="sb", bufs=1))
        xs = ctx.enter_context(tc.tile_pool(name="xs", bufs=4))
        exp = ctx.enter_context(tc.tile_pool(name="exp", bufs=2))
        ob = ctx.enter_context(tc.tile_pool(name="ob", bufs=2))
        ps = ctx.enter_context(tc.tile_pool(name="ps", bufs=2, space="PSUM"))

        # --- weights, resident (wk first: first needed) ---
        w_sb = {}
        for name, w in (("wk", wk), ("wv", wv), ("wq", wq)):
            t = sb.tile([128, KC, FPC], BF16, name=f"{name}_sb", tag=f"{name}_sb")
            nc.sync.dma_start(out=t, in_=w.rearrange("(kc p) f -> p kc f", p=128))
            w_sb[name] = t

        # --- persistent projection outputs, one tile per s-chunk ---
        qT = [sb.tile([128, 4, J], BF16, name=f"qT{j}", tag=f"qT{j}")
              for j in range(NJ)]   # [f%128, f//128, s in chunk j]
        kT = [sb.tile([128, 4, J], BF16, name=f"kT{j}", tag=f"kT{j}")
              for j in range(NJ)]
        vaug = sb.tile([128, NT, HPC, D + 1], BF16, name="vaug", tag="vaug")
        nc.vector.memset(vaug, 1.0)   # col D stays 1; cols 0..D-1 overwritten

        for r in range(repeat):
            emit_body(nc, tc, xq, xk, xv, out, w_sb, qT, kT, vaug,
                      xs, exp, ob, ps, r)

    nc.compile()
    nc.freeze()
    return nc


def emit_body(nc, tc, xq, xk, xv, out, w_sb, qT, kT, vaug, xs, exp, ob, ps, r):
    def load_x(name, x, j):
        xt = xs.tile([128, KC, J], BF16, name=f"x_{name}_{j}_{r}", tag="xt")
        nc.sync.dma_start(
            out=xt,
            in_=x[:, j * J:(j + 1) * J].rearrange("(kc p) f -> p kc f", p=128),
        )
        return xt

    def proj_fmajor(name, xt, dst, j, m):
        # dst[j][:, m, :] = (W[:, 128m:128(m+1)])^T @ x[:, Jj:J(j+1)]
        acc = ps.tile([128, J], F32, name=f"p_{name}_{j}_{m}_{r}", tag="acc")
        for kc in range(KC):
            nc.tensor.matmul(
                acc,
                lhsT=w_sb[name][:, kc, m * 128:(m + 1) * 128],
                rhs=xt[:, kc, :],
                start=(kc == 0), stop=(kc == KC - 1),
            )
        nc.vector.tensor_copy(out=dst[j][:, m, :], in_=acc)

    def proj_v(xt, j, sc):
        t = j * (J // 128) + sc
        acc = ps.tile([128, FPC], F32, name=f"p_v_{j}_{sc}_{r}", tag="acc")
        for kc in range(KC):
            nc.tensor.matmul(
                acc,
                lhsT=xt[:, kc, sc * 128:(sc + 1) * 128],
                rhs=w_sb["wv"][:, kc, :],
                start=(kc == 0), stop=(kc == KC - 1),
            )
        nc.vector.tensor_copy(
            out=vaug[:, t, :, 0:D],
            in_=acc.rearrange("p (h d) -> p h d", h=HPC))

    def attention(h, j, ostage, tgrange, cacc=None):
        po = (h % 2) * 64   # partition offset of head h inside its chunk
        m = h // 2
        if cacc is None:
            cacc = ps.tile([D + 1, J], F32, name=f"ctx_{h}_{j}_{r}", tag="acc")
        for tg in tgrange:
            st = ps.tile([128, 2, J], F32, name=f"st_{h}_{j}_{tg}_{r}",
                         tag="st", bufs=3)
            for u in range(2):
                t = 2 * tg + u
                nc.tensor.matmul(
                    st[:, u, :],
                    lhsT=kT[t // 4][po:po + 64, m,
                                    (t % 4) * 128:(t % 4) * 128 + 128],
                    rhs=qT[j][po:po + 64, m, :],
                    start=True, stop=True,
                )
            ex = exp.tile([128, 2, J], BF16, name=f"ex_{h}_{j}_{tg}_{r}",
                          tag="ex", bufs=4)
            nc.scalar.activation(ex, st, EXP, scale=SCALE)
            for u in range(2):
                t = 2 * tg + u
                nc.tensor.matmul(
                    cacc,
                    lhsT=vaug[:, t, h, :],
                    rhs=ex[:, u, :],
                    start=(t == 0), stop=(t == NT - 1),
                )
        if tgrange[-1] == NT // 2 - 1:
            nc.vector.tensor_copy(out=ostage[:, h, :], in_=cacc)
        return cacc

    def dma_out(ostage, j, p):   # heads 2p, 2p+1 of chunk j
        nc.sync.dma_start(
            out=out.rearrange("h d s -> d h s")[:, 2 * p:2 * p + 2,
                                                j * J:(j + 1) * J],
            in_=ostage[:, 2 * p:2 * p + 2, :])

    # --- j0 projections; attention h0 split 4 ways over the remaining
    #     K/V chunk loads; Q chains of chunk j+1 as filler between pairs ---
    xk0 = load_x("wk", xk, 0)
    for m in range(4):
        proj_fmajor("wk", xk0, kT, 0, m)
    xv0 = load_x("wv", xv, 0)
    for sc in range(4):
        proj_v(xv0, 0, sc)
    xq0 = load_x("wq", xq, 0)
    for m in range(4):
        proj_fmajor("wq", xq0, qT, 0, m)

    ost = [None] * NJ
    ost[0] = ob.tile([D + 1, HPC, J], F32, name=f"ost_0_{r}", tag="ost")
    cacc0 = attention(0, 0, ost[0], range(0, 2))
    for jj in range(1, 4):
        xkj = load_x("wk", xk, jj)
        for m in range(4):
            proj_fmajor("wk", xkj, kT, jj, m)
        xvj = load_x("wv", xv, jj)
        for sc in range(4):
            proj_v(xvj, jj, sc)
        attention(0, 0, ost[0], range(2 * jj, 2 * jj + 2), cacc=cacc0)
    attention(1, 0, ost[0], range(NT // 2))
    dma_out(ost[0], 0, 0)

    xqj = load_x("wq", xq, 1)
    for p in range(1, 4):
        proj_fmajor("wq", xqj, qT, 1, p - 1)
        attention(2 * p, 0, ost[0], range(NT // 2))
        attention(2 * p + 1, 0, ost[0], range(NT // 2))
        dma_out(ost[0], 0, p)
    proj_fmajor("wq", xqj, qT, 1, 3)

    for j in range(1, NJ):
        ost[j] = ob.tile([D + 1, HPC, J], F32, name=f"ost_{j}_{r}", tag="ost")
        if j + 1 < NJ:
            xqj = load_x("wq", xq, j + 1)
        for p in range(4):
            if j + 1 < NJ:
                proj_fmajor("wq", xqj, qT, j + 1, p)
            attention(2 * p, j, ost[j], range(NT // 2))
            attention(2 * p + 1, j, ost[j], range(NT // 2))
            dma_out(ost[j], j, p)


_NC_CACHE = None


def _get_nc():
    global _NC_CACHE
    if _NC_CACHE is None:
        _NC_CACHE = build_bass()
    return _NC_CACHE


def make_in_maps(queries, keys, values, Wq, Wk, Wv):
    # Host-side shard prep: cast to bf16, transpose activations per batch,
    # slice W column-wise by head group.
    try:
        import ml_dtypes
        bf16 = ml_dtypes.bfloat16
    except ImportError:  # pragma: no cover
        import jax.numpy as jnp
        bf16 = jnp.bfloat16
    xq_t = [np.ascontiguousarray(queries[b].T).astype(bf16) for b in range(B)]
    xk_t = [np.ascontiguousarray(keys[b].T).astype(bf16) for b in range(B)]
    xv_t = [np.ascontiguousarray(values[b].T).astype(bf16) for b in range(B)]
    w_half = [
        (np.ascontiguousarray(Wq[:, g * FPC:(g + 1) * FPC]).astype(bf16),
         np.ascontiguousarray(Wk[:, g * FPC:(g + 1) * FPC]).astype(bf16),
         np.ascontiguousarray(Wv[:, g * FPC:(g + 1) * FPC]).astype(bf16))
        for g in range(2)
    ]

    in_maps = []
    for c in range(N_CORES):
        b, g = c // 2, c % 2
        in_maps.append({
            "xq_t": xq_t[b], "xk_t": xk_t[b], "xv_t": xv_t[b],
            "wq": w_half[g][0], "wk": w_half[g][1], "wv": w_half[g][2],
        })
    return in_maps


def kernel(queries, keys, values, Wq, Wk, Wv, **_):
    queries = np.asarray(queries, dtype=np.float32)
    keys = np.asarray(keys, dtype=np.float32)
    values = np.asarray(values, dtype=np.float32)
    Wq = np.asarray(Wq, dtype=np.float32)
    Wk = np.asarray(Wk, dtype=np.float32)
    Wv = np.asarray(Wv, dtype=np.float32)

    in_maps = make_in_maps(queries, keys, values, Wq, Wk, Wv)

    nc = _get_nc()
    res = run_bass_kernel_spmd(nc, in_maps, list(range(N_CORES)))

    full = np.empty((B, S, H * D), dtype=np.float32)
    for c in range(N_CORES):
        b, g = c // 2, c % 2
        o = res.results[c]["out"]          # [HPC, D+1, S]
        ctx = o[:, :D, :] / o[:, D:D + 1, :]     # [HPC, D, S]
        dst = full[b].reshape(S, H, D)
        dst[:, g * HPC:(g + 1) * HPC, :] = ctx.transpose(2, 0, 1)
    return full


# revision 10
# speedup vs baseline: 1.4029x; 1.1762x over previous
"""Multi-head attention (B=4, S=2048, E=1024, H=16, D=64) on 8 TRN2 NeuronCores.

Sharding: core c handles batch b = c//2 and heads [8*(c%2), 8*(c%2)+8) —
data parallel over batch, tensor parallel over heads. No collectives.

v5: 512-wide moving operands (ISA limit s3d3_mm_num_elements), bf16
operands (half the HBM traffic / SBUF / host transfer; PSUM accumulation
fp32), PSUM: st pairs triple-buffered (6 banks) + one shared 2-slot acc
tag (cacc + projection accumulators), ACT over 2-bank pairs (256
activations), emission order = schedule priority: j0 projections first,
head 0's t-loop split 4 ways so K/V projections of later j-chunks are
emitted exactly where needed, Q projections of chunk j+1 spread between
attention head pairs as PE filler, per-2-head batched output DMAs.
"""

import numpy as np
from contextlib import ExitStack

import concourse.bass as bass
import concourse.tile as tile
from concourse import bacc
from concourse import mybir
from concourse.bass_utils import run_bass_kernel_spmd

F32 = mybir.dt.float32
BF16 = mybir.dt.bfloat16
EXP = mybir.ActivationFunctionType.Exp

B, S, E = 4, 2048, 1024
H, D = 16, 64
HPC = 8            # heads per core
FPC = HPC * D      # 512 output features per core
N_CORES = 8
KC = E // 128      # contraction chunks
J = 512            # s chunk width
NJ = S // J        # s chunks (4)
NT = S // 128      # sk blocks (16)
SCALE = 0.125      # 1/sqrt(64)


def build_bass(repeat=1):
    nc = bacc.Bacc()
    xq = nc.declare_dram_parameter("xq_t", [E, S], BF16, isOutput=False)
    xk = nc.declare_dram_parameter("xk_t", [E, S], BF16, isOutput=False)
    xv = nc.declare_dram_parameter("xv_t", [E, S], BF16, isOutput=False)
    wq = nc.declare_dram_parameter("wq", [E, FPC], BF16, isOutput=False)
    wk = nc.declare_dram_parameter("wk", [E, FPC], BF16, isOutput=False)
    wv = nc.declare_dram_parameter("wv", [E, FPC], BF16, isOutput=False)
    out = nc.declare_dram_parameter("out", [HPC, D + 1, S], F32, isOutput=True)

    with tile.TileContext(nc) as tc, ExitStack() as ctx:
        sb = ctx.enter_context(tc.tile_pool(name="sb", bufs=1))
        xs = ctx.enter_context(tc.tile_pool(name="xs", bufs=4))
        exp = ctx.enter_context(tc.tile_pool(name="exp", bufs=2))
        ob = ctx.enter_context(tc.tile_pool(name="ob", bufs=2))
        ps = ctx.enter_context(tc.tile_pool(name="ps", bufs=2, space="PSUM"))

        # --- weights, resident; only wk loaded up front, wv/wq are
        #     DMA'd from the body in consumer order (r==0) ---
        w_sb = {}
        w_dram = {"wk": wk, "wv": wv, "wq": wq}
        for name in ("wk", "wv", "wq"):
            w_sb[name] = sb.tile([128, KC, FPC], BF16, name=f"{name}_sb",
                                 tag=f"{name}_sb")
        nc.sync.dma_start(
            out=w_sb["wk"],
            in_=wk.rearrange("(kc p) f -> p kc f", p=128))

        # --- persistent projection outputs, one tile per s-chunk ---
        qT = [sb.tile([128, 4, J], BF16, name=f"qT{j}", tag=f"qT{j}")
              for j in range(NJ)]   # [f%128, f//128, s in chunk j]
        kT = [sb.tile([128, 4, J], BF16, name=f"kT{j}", tag=f"kT{j}")
              for j in range(NJ)]
        vaug = sb.tile([128, NT, HPC, D + 1], BF16, name="vaug", tag="vaug")
        nc.vector.memset(vaug, 1.0)   # col D stays 1; cols 0..D-1 overwritten

        for r in range(repeat):
            emit_body(nc, tc, xq, xk, xv, out, w_sb, qT, kT, vaug,
                      xs, exp, ob, ps, r, w_dram=w_dram)

    nc.compile()
    nc.freeze()
    return nc


def emit_body(nc, tc, xq, xk, xv, out, w_sb, qT, kT, vaug, xs, exp, ob, ps, r,
              w_dram=None):
    def load_x(name, x, j):
        xt = xs.tile([128, KC, J], BF16, name=f"x_{name}_{j}_{r}", tag="xt")
        src_v = x[:, j * J:(j + 1) * J].rearrange("(kc p) f -> p kc f", p=128)
        h = KC // 2
        nc.sync.dma_start(out=xt[:, 0:h], in_=src_v[:, 0:h])
        nc.sync.dma_start(out=xt[:, h:KC], in_=src_v[:, h:KC])
        return xt

    def proj_fmajor(name, xt, dst, j, m):
        # dst[j][:, m, :] = (W[:, 128m:128(m+1)])^T @ x[:, Jj:J(j+1)]
        acc = ps.tile([128, J], F32, name=f"p_{name}_{j}_{m}_{r}", tag="st",
                      bufs=3)
        for kc in range(KC):
            nc.tensor.matmul(
                acc,
                lhsT=w_sb[name][:, kc, m * 128:(m + 1) * 128],
                rhs=xt[:, kc, :],
                start=(kc == 0), stop=(kc == KC - 1),
            )
        nc.vector.tensor_copy(out=dst[j][:, m, :], in_=acc)

    def proj_v(xt, j, sc):
        t = j * (J // 128) + sc
        acc = ps.tile([128, FPC], F32, name=f"p_v_{j}_{sc}_{r}", tag="st",
                      bufs=3)
        for kc in range(KC):
            nc.tensor.matmul(
                acc,
                lhsT=xt[:, kc, sc * 128:(sc + 1) * 128],
                rhs=w_sb["wv"][:, kc, :],
                start=(kc == 0), stop=(kc == KC - 1),
            )
        nc.vector.tensor_copy(
            out=vaug[:, t, :, 0:D],
            in_=acc.rearrange("p (h d) -> p h d", h=HPC))

    def scores(h, j, tg):
        po = (h % 2) * 64   # partition offset of head h inside its chunk
        m = h // 2
        st = ps.tile([128, 2, J], F32, name=f"st_{h}_{j}_{tg}_{r}",
                     tag="st", bufs=3)
        for u in range(2):
            t = 2 * tg + u
            nc.tensor.matmul(
                st[:, u, :],
                lhsT=kT[t // 4][po:po + 64, m,
                                (t % 4) * 128:(t % 4) * 128 + 128],
                rhs=qT[j][po:po + 64, m, :],
                start=True, stop=True,
            )
        return st

    def softmax_ctx(h, j, tg, st, cacc, start, stop):
        ex = exp.tile([128, 2, J], BF16, name=f"ex_{h}_{j}_{tg}_{r}",
                      tag="ex", bufs=4)
        nc.scalar.activation(ex, st, EXP, scale=SCALE)
        for u in range(2):
            t = 2 * tg + u
            nc.tensor.matmul(
                cacc,
                lhsT=vaug[:, t, h, :],
                rhs=ex[:, u, :],
                start=(start and u == 0), stop=(stop and u == 1),
            )

    def dma_out(ostage, j, p):   # heads 2p, 2p+1 of chunk j
        nc.sync.dma_start(
            out=out.rearrange("h d s -> d h s")[:, 2 * p:2 * p + 2,
                                                j * J:(j + 1) * J],
            in_=ostage[:, 2 * p:2 * p + 2, :])

    # ---- unit-stream pipeline: scores run 2 units ahead of exp/ctx so
    #      ACT never waits at head boundaries or across filler chains ----
    units = []    # (h, j, tg, first, last, after_thunks, barrier)

    def unit(h, j, tg, first, last, after=(), barrier=False):
        units.append((h, j, tg, first, last, tuple(after), barrier))

    caccs = {}
    ost = [None] * NJ

    def run_units():
        sts = {}
        n = len(units)

        def emit_scores(i):
            h, j, tg = units[i][:3]
            sts[i] = scores(h, j, tg)

        emit_scores(0)
        if n > 1:
            emit_scores(1)
        for i in range(n):
            h, j, tg, first, last, after, barrier = units[i]
            if not barrier and i + 2 < n:
                emit_scores(i + 2)
            if first:
                caccs[(h, j)] = ps.tile([D + 1, J], F32,
                                        name=f"ctx_{h}_{j}_{r}", tag="acc")
            softmax_ctx(h, j, tg, sts.pop(i), caccs[(h, j)], first, last)
            if last:
                nc.vector.tensor_copy(out=ost[j][:, h, :],
                                      in_=caccs.pop((h, j)))
            for thunk in after:
                thunk()
            # barrier unit: lookahead scores only after the thunks so a
            # K/V chain emitted here is in program order before any score
            # that reads what it writes
            if barrier and i + 2 < n:
                emit_scores(i + 2)
        units.clear()

    # ---- j0: prologue projections, h0+h1 wavefront over K/V chunk loads,
    #      then heads 2..7; Q chains of chunk j+1 interleaved as filler ----
    xk0 = load_x("wk", xk, 0)
    if r == 0:
        nc.sync.dma_start(
            out=w_sb["wv"],
            in_=w_dram["wv"].rearrange("(kc p) f -> p kc f", p=128))
    xv0 = load_x("wv", xv, 0)
    if r == 0:
        nc.sync.dma_start(
            out=w_sb["wq"],
            in_=w_dram["wq"].rearrange("(kc p) f -> p kc f", p=128))
    xq0 = load_x("wq", xq, 0)
    for m in range(4):
        proj_fmajor("wk", xk0, kT, 0, m)
    for sc in range(4):
        proj_v(xv0, 0, sc)
    for m in range(4):
        proj_fmajor("wq", xq0, qT, 0, m)
    ost[0] = ob.tile([D + 1, HPC, J], F32, name=f"ost_0_{r}", tag="ost")

    xkv = {1: (load_x("wk", xk, 1), load_x("wv", xv, 1))}

    def kv_chunk(jj, half):
        # half 0: K chains; half 1: V chains + prefetch of chunk jj+1
        def thunk():
            if half == 0:
                for m in range(4):
                    proj_fmajor("wk", xkv[jj][0], kT, jj, m)
            else:
                for sc in range(4):
                    proj_v(xkv[jj][1], jj, sc)
                if jj + 1 < 4:
                    xkv[jj + 1] = (load_x("wk", xk, jj + 1),
                                   load_x("wv", xv, jj + 1))
        return thunk

    unit(0, 0, 0, True, False)
    unit(1, 0, 0, True, False)
    unit(0, 0, 1, False, False, after=[kv_chunk(1, 0)], barrier=True)
    unit(1, 0, 1, False, False, after=[kv_chunk(1, 1)], barrier=True)
    for jj in range(1, 4):
        unit(0, 0, 2 * jj, False, False)
        unit(1, 0, 2 * jj, False, False)
        unit(0, 0, 2 * jj + 1, False, jj == 3,
             after=[kv_chunk(jj + 1, 0)] if jj < 3 else (), barrier=jj < 3)
        unit(1, 0, 2 * jj + 1, False, jj == 3,
             after=[kv_chunk(jj + 1, 1)] if jj < 3 else (), barrier=jj < 3)
    run_units()
    dma_out(ost[0], 0, 0)

    xq_t = {1: load_x("wq", xq, 1)}

    def q_chain(j, m):
        def thunk():
            proj_fmajor("wq", xq_t[j], qT, j, m)
        return thunk

    def q_load(j):
        def thunk():
            xq_t[j] = load_x("wq", xq, j)
        return thunk

    for p in range(1, 4):
        for tg in range(NT // 2):
            for h in (2 * p, 2 * p + 1):
                after = []
                if h == 2 * p and tg == 0:
                    after.append(q_chain(1, p - 1))
                if h == 2 * p + 1 and tg == NT // 2 - 1:
                    after.append(lambda p=p: dma_out(ost[0], 0, p))
                unit(h, 0, tg, tg == 0, tg == NT // 2 - 1, after)
    run_units()

    for j in range(1, NJ):
        ost[j] = ob.tile([D + 1, HPC, J], F32, name=f"ost_{j}_{r}", tag="ost")
        for p in range(4):
            for tg in range(NT // 2):
                for h in (2 * p, 2 * p + 1):
                    after = []
                    if h == 2 * p and tg == 0:
                        if p == 0 and j == 1:
                            after.append(q_chain(1, 3))
                        if j + 1 < NJ:
                            if p == 0:
                                after.append(q_load(j + 1))
                            after.append(q_chain(j + 1, p))
                    if h == 2 * p + 1 and tg == NT // 2 - 1:
                        after.append(lambda j=j, p=p: dma_out(ost[j], j, p))
                    unit(h, j, tg, tg == 0, tg == NT // 2 - 1, after)
        run_units()


_NC_CACHE = None


def _get_nc():
    global _NC_CACHE
    if _NC_CACHE is None:
        _NC_CACHE = build_bass()
    return _NC_CACHE


def make_in_maps(queries, keys, values, Wq, Wk, Wv):
    # Host-side shard prep: cast to bf16, transpose activations per batch,
    # slice W column-wise by head group.
    try:
        import ml_dtypes
        bf16 = ml_dtypes.bfloat16
    except ImportError:  # pragma: no cover
        import jax.numpy as jnp
        bf16 = jnp.bfloat16
    xq_t = [np.ascontiguousarray(queries[b].T).astype(bf16) for b in range(B)]
    xk_t = [np.ascontiguousarray(keys[b].T).astype(bf16) for b in range(B)]
    xv_t = [np.ascontiguousarray(values[b].T).astype(bf16) for b in range(B)]
    w_half = [
        (np.ascontiguousarray(Wq[:, g * FPC:(g + 1) * FPC]).astype(bf16),
         np.ascontiguousarray(Wk[:, g * FPC:(g + 1) * FPC]).astype(bf16),
         np.ascontiguousarray(Wv[:, g * FPC:(g + 1) * FPC]).astype(bf16))
        for g in range(2)
    ]

    in_maps = []
    for c in range(N_CORES):
        b, g = c // 2, c % 2
        in_maps.append({
            "xq_t": xq_t[b], "xk_t": xk_t[b], "xv_t": xv_t[b],
            "wq": w_half[g][0], "wk": w_half[g][1], "wv": w_half[g][2],
        })
    return in_maps


def kernel(queries, keys, values, Wq, Wk, Wv, **_):
    queries = np.asarray(queries, dtype=np.float32)
    keys = np.asarray(keys, dtype=np.float32)
    values = np.asarray(values, dtype=np.float32)
    Wq = np.asarray(Wq, dtype=np.float32)
    Wk = np.asarray(Wk, dtype=np.float32)
    Wv = np.asarray(Wv, dtype=np.float32)

    in_maps = make_in_maps(queries, keys, values, Wq, Wk, Wv)

    nc = _get_nc()
    res = run_bass_kernel_spmd(nc, in_maps, list(range(N_CORES)))

    full = np.empty((B, S, H * D), dtype=np.float32)
    for c in range(N_CORES):
        b, g = c // 2, c % 2
        o = res.results[c]["out"]          # [HPC, D+1, S]
        ctx = o[:, :D, :] / o[:, D:D + 1, :]     # [HPC, D, S]
        dst = full[b].reshape(S, H, D)
        dst[:, g * HPC:(g + 1) * HPC, :] = ctx.transpose(2, 0, 1)
    return full


# revision 13
# speedup vs baseline: 1.7091x; 1.2182x over previous
"""Multi-head attention (B=4, S=2048, E=1024, H=16, D=64) on 8 TRN2 NeuronCores.

Sharding: core c handles batch b = c//2 and heads [8*(c%2), 8*(c%2)+8) —
data parallel over batch, tensor parallel over heads. No collectives.

Design (cost-model-guided; TimelineSim ~351us/core vs 417us for the
fp32r baseline; rel err vs fp32 reference ~7.5e-3):
  - 512-wide moving operands (walrus ISA cap s3d3_mm_num_elements); all
    matmul operands bf16 (PSUM accumulation stays fp32): halves HBM
    traffic, SBUF footprint and host->device transfer.
  - Softmax exp runs on ACT from PSUM in ragged 3-block groups
    ([3,3,3,3,2,2] sk-blocks per head) - 208 activations instead of 256,
    amortizing the fixed per-instruction ACT overhead that paces the
    steady state. PSUM: 3-bank score slots (2 bufs, shared with
    projection accumulators) + 2 banks of ctx accumulators.
  - Unit-stream pipeline: score matmuls are emitted two units ahead of
    exp/ctx so ACT never head-blocks at head boundaries/filler chains.
    Head pairs interleave per score group so adjacent K=64 score
    matmuls target opposite PE row-halves (tile_position 0/64) and can
    overlap via per-subarray concurrency on hardware.
  - j0 is a wavefront: heads 0-1 advance two sk-blocks per K/V chunk
    projection phase, remaining heads follow; Q projections of chunk
    j+1 interleave into chunk j's attention as PE filler. DMAs are
    emitted in consumer order; x tiles load in kc-halves; output DMAs
    batched per head-pair (final pair per-head) as soon as they finish.
"""

import numpy as np
from contextlib import ExitStack

import concourse.bass as bass
import concourse.tile as tile
from concourse import bacc
from concourse import mybir
from concourse.bass_utils import run_bass_kernel_spmd

F32 = mybir.dt.float32
BF16 = mybir.dt.bfloat16
EXP = mybir.ActivationFunctionType.Exp

B, S, E = 4, 2048, 1024
H, D = 16, 64
HPC = 8            # heads per core
FPC = HPC * D      # 512 output features per core
N_CORES = 8
KC = E // 128      # contraction chunks
J = 512            # s chunk width
NJ = S // J        # s chunks (4)
NT = S // 128      # sk blocks (16)
SCALE = 0.125      # 1/sqrt(64)


def build_bass(repeat=1):
    nc = bacc.Bacc()
    xq = nc.declare_dram_parameter("xq_t", [E, S], BF16, isOutput=False)
    xk = nc.declare_dram_parameter("xk_t", [E, S], BF16, isOutput=False)
    xv = nc.declare_dram_parameter("xv_t", [E, S], BF16, isOutput=False)
    wq = nc.declare_dram_parameter("wq", [E, FPC], BF16, isOutput=False)
    wk = nc.declare_dram_parameter("wk", [E, FPC], BF16, isOutput=False)
    wv = nc.declare_dram_parameter("wv", [E, FPC], BF16, isOutput=False)
    out = nc.declare_dram_parameter("out", [HPC, D + 1, S], F32, isOutput=True)

    with tile.TileContext(nc) as tc, ExitStack() as ctx:
        sb = ctx.enter_context(tc.tile_pool(name="sb", bufs=1))
        xs = ctx.enter_context(tc.tile_pool(name="xs", bufs=4))
        exp = ctx.enter_context(tc.tile_pool(name="exp", bufs=2))
        ob = ctx.enter_context(tc.tile_pool(name="ob", bufs=2))
        ps = ctx.enter_context(tc.tile_pool(name="ps", bufs=2, space="PSUM"))

        # --- weights, resident; only wk loaded up front, wv/wq are
        #     DMA'd from the body in consumer order (r==0) ---
        w_sb = {}
        w_dram = {"wk": wk, "wv": wv, "wq": wq}
        for name in ("wk", "wv", "wq"):
            w_sb[name] = sb.tile([128, KC, FPC], BF16, name=f"{name}_sb",
                                 tag=f"{name}_sb")
        nc.sync.dma_start(
            out=w_sb["wk"],
            in_=wk.rearrange("(kc p) f -> p kc f", p=128))

        # --- persistent projection outputs, one tile per s-chunk ---
        qT = [sb.tile([128, 4, J], BF16, name=f"qT{j}", tag=f"qT{j}")
              for j in range(NJ)]   # [f%128, f//128, s in chunk j]
        kT = [sb.tile([128, 4, J], BF16, name=f"kT{j}", tag=f"kT{j}")
              for j in range(NJ)]
        vaug = sb.tile([128, NT, HPC, D + 1], BF16, name="vaug", tag="vaug")
        nc.vector.memset(vaug, 1.0)   # col D stays 1; cols 0..D-1 overwritten

        for r in range(repeat):
            emit_body(nc, tc, xq, xk, xv, out, w_sb, qT, kT, vaug,
                      xs, exp, ob, ps, r, w_dram=w_dram)

    nc.compile()
    nc.freeze()
    return nc


def emit_body(nc, tc, xq, xk, xv, out, w_sb, qT, kT, vaug, xs, exp, ob, ps, r,
              w_dram=None):
    def load_x(name, x, j):
        xt = xs.tile([128, KC, J], BF16, name=f"x_{name}_{j}_{r}", tag="xt")
        src_v = x[:, j * J:(j + 1) * J].rearrange("(kc p) f -> p kc f", p=128)
        h = KC // 2
        nc.sync.dma_start(out=xt[:, 0:h], in_=src_v[:, 0:h])
        nc.sync.dma_start(out=xt[:, h:KC], in_=src_v[:, h:KC])
        return xt

    def proj_fmajor(name, xt, dst, j, m):
        # dst[j][:, m, :] = (W[:, 128m:128(m+1)])^T @ x[:, Jj:J(j+1)]
        acc = ps.tile([128, J], F32, name=f"p_{name}_{j}_{m}_{r}", tag="st",
                      bufs=2, padded_shape=[128, 3 * J])
        for kc in range(KC):
            nc.tensor.matmul(
                acc,
                lhsT=w_sb[name][:, kc, m * 128:(m + 1) * 128],
                rhs=xt[:, kc, :],
                start=(kc == 0), stop=(kc == KC - 1),
            )
        nc.vector.tensor_copy(out=dst[j][:, m, :], in_=acc)

    def proj_v(xt, j, sc):
        t = j * (J // 128) + sc
        acc = ps.tile([128, FPC], F32, name=f"p_v_{j}_{sc}_{r}", tag="st",
                      bufs=2, padded_shape=[128, 3 * J])
        for kc in range(KC):
            nc.tensor.matmul(
                acc,
                lhsT=xt[:, kc, sc * 128:(sc + 1) * 128],
                rhs=w_sb["wv"][:, kc, :],
                start=(kc == 0), stop=(kc == KC - 1),
            )
        nc.vector.tensor_copy(
            out=vaug[:, t, :, 0:D],
            in_=acc.rearrange("p (h d) -> p h d", h=HPC))

    def scores(h, j, tset):
        po = (h % 2) * 64   # partition offset of head h inside its chunk
        m = h // 2
        st = ps.tile([128, len(tset), J], F32,
                     name=f"st_{h}_{j}_{tset[0]}_{r}", tag="st", bufs=2,
                     padded_shape=[128, 3, J])
        for u, t in enumerate(tset):
            nc.tensor.matmul(
                st[:, u, :],
                lhsT=kT[t // 4][po:po + 64, m,
                                (t % 4) * 128:(t % 4) * 128 + 128],
                rhs=qT[j][po:po + 64, m, :],
                start=True, stop=True,
            )
        return st

    def softmax_ctx(h, j, tset, st, cacc, start, stop):
        ex = exp.tile([128, len(tset), J], BF16,
                      name=f"ex_{h}_{j}_{tset[0]}_{r}", tag="ex", bufs=6,
                      padded_shape=[128, 3, J])
        nc.scalar.activation(ex, st, EXP, scale=SCALE)
        for u, t in enumerate(tset):
            nc.tensor.matmul(
                cacc,
                lhsT=vaug[:, t, h, :],
                rhs=ex[:, u, :],
                start=(start and u == 0),
                stop=(stop and u == len(tset) - 1),
            )

    def dma_out(ostage, j, p):   # heads 2p, 2p+1 of chunk j
        nc.sync.dma_start(
            out=out.rearrange("h d s -> d h s")[:, 2 * p:2 * p + 2,
                                                j * J:(j + 1) * J],
            in_=ostage[:, 2 * p:2 * p + 2, :])

    # ---- unit-stream pipeline: scores run 2 units ahead of exp/ctx so
    #      ACT never waits at head boundaries or across filler chains ----
    units = []    # (h, j, tset, first, last, after_thunks, barrier)

    def unit(h, j, tg, first, last, after=(), barrier=False, tset=None):
        if tset is None:
            tset = [2 * tg, 2 * tg + 1]
        units.append((h, j, tset, first, last, tuple(after), barrier))

    caccs = {}
    ost = [None] * NJ

    def run_units():
        sts = {}
        n = len(units)

        def emit_scores(i):
            h, j, tset = units[i][:3]
            sts[i] = scores(h, j, tset)

        emit_scores(0)
        if n > 1:
            emit_scores(1)
        for i in range(n):
            h, j, tset, first, last, after, barrier = units[i]
            if not barrier and i + 2 < n:
                emit_scores(i + 2)
            if first:
                caccs[(h, j)] = ps.tile([D + 1, J], F32,
                                        name=f"ctx_{h}_{j}_{r}", tag="acc")
            softmax_ctx(h, j, tset, sts.pop(i), caccs[(h, j)], first, last)
            if last:
                nc.vector.tensor_copy(out=ost[j][:, h, :],
                                      in_=caccs.pop((h, j)))
            for thunk in after:
                thunk()
            # barrier unit: lookahead scores only after the thunks so a
            # K/V chain emitted here is in program order before any score
            # that reads what it writes
            if barrier and i + 2 < n:
                emit_scores(i + 2)
        units.clear()

    # ---- j0: prologue projections, h0+h1 wavefront over K/V chunk loads,
    #      then heads 2..7; Q chains of chunk j+1 interleaved as filler ----
    xk0 = load_x("wk", xk, 0)
    if r == 0:
        nc.sync.dma_start(
            out=w_sb["wv"],
            in_=w_dram["wv"].rearrange("(kc p) f -> p kc f", p=128))
    xv0 = load_x("wv", xv, 0)
    if r == 0:
        nc.sync.dma_start(
            out=w_sb["wq"],
            in_=w_dram["wq"].rearrange("(kc p) f -> p kc f", p=128))
    xq0 = load_x("wq", xq, 0)
    for m in range(4):
        proj_fmajor("wk", xk0, kT, 0, m)
    for sc in range(4):
        proj_v(xv0, 0, sc)
    for m in range(4):
        proj_fmajor("wq", xq0, qT, 0, m)
    ost[0] = ob.tile([D + 1, HPC, J], F32, name=f"ost_0_{r}", tag="ost")

    xkv = {1: (load_x("wk", xk, 1), load_x("wv", xv, 1))}

    def kv_chunk(jj, half):
        # half 0: K chains; half 1: V chains + prefetch of chunk jj+1
        def thunk():
            if half == 0:
                for m in range(4):
                    proj_fmajor("wk", xkv[jj][0], kT, jj, m)
            else:
                for sc in range(4):
                    proj_v(xkv[jj][1], jj, sc)
                if jj + 1 < 4:
                    xkv[jj + 1] = (load_x("wk", xk, jj + 1),
                                   load_x("wv", xv, jj + 1))
        return thunk

    unit(0, 0, 0, True, False)
    unit(1, 0, 0, True, False)
    unit(0, 0, 1, False, False, after=[kv_chunk(1, 0)], barrier=True)
    unit(1, 0, 1, False, False, after=[kv_chunk(1, 1)], barrier=True)
    for jj in range(1, 4):
        unit(0, 0, 2 * jj, False, False)
        unit(1, 0, 2 * jj, False, False)
        unit(0, 0, 2 * jj + 1, False, jj == 3,
             after=[kv_chunk(jj + 1, 0)] if jj < 3 else (), barrier=jj < 3)
        unit(1, 0, 2 * jj + 1, False, jj == 3,
             after=[kv_chunk(jj + 1, 1)] if jj < 3 else (), barrier=jj < 3)
    run_units()
    dma_out(ost[0], 0, 0)

    xq_t = {1: load_x("wq", xq, 1)}

    def q_chain(j, m):
        def thunk():
            proj_fmajor("wq", xq_t[j], qT, j, m)
        return thunk

    def q_load(j):
        def thunk():
            xq_t[j] = load_x("wq", xq, j)
        return thunk

    for p in range(1, 4):
        for tg in range(NT // 2):
            for h in (2 * p, 2 * p + 1):
                after = []
                if h == 2 * p and tg == 0:
                    after.append(q_chain(1, p - 1))
                if h == 2 * p + 1 and tg == NT // 2 - 1:
                    after.append(lambda p=p: dma_out(ost[0], 0, p))
                unit(h, 0, tg, tg == 0, tg == NT // 2 - 1, after)

    for j in range(1, NJ):
        ost[j] = ob.tile([D + 1, HPC, J], F32, name=f"ost_{j}_{r}", tag="ost")
        TSETS = [[0, 1, 2], [3, 4, 5], [6, 7, 8], [9, 10, 11],
                 [12, 13], [14, 15]]
        for p in range(4):
            for tg, ts_ in enumerate(TSETS):
                for h in (2 * p, 2 * p + 1):
                    after = []
                    if h == 2 * p and tg == 0:
                        if p == 0 and j == 1:
                            after.append(q_chain(1, 3))
                        if j + 1 < NJ:
                            if p == 0:
                                after.append(q_load(j + 1))
                            after.append(q_chain(j + 1, p))
                    if h == 2 * p + 1 and tg == len(TSETS) - 1:
                        if j == NJ - 1 and p == 3:
                            after.append(lambda: nc.sync.dma_start(
                                out=out.rearrange("h d s -> d h s")[
                                    :, 7:8, (NJ - 1) * J:NJ * J],
                                in_=ost[NJ - 1][:, 7:8, :]))
                        else:
                            after.append(lambda j=j, p=p: dma_out(ost[j], j, p))
                    if h == 2 * p and tg == len(TSETS) - 1 and j == NJ - 1 \
                            and p == 3:
                        after.append(lambda: nc.sync.dma_start(
                            out=out.rearrange("h d s -> d h s")[
                                :, 6:7, (NJ - 1) * J:NJ * J],
                            in_=ost[NJ - 1][:, 6:7, :]))
                    unit(h, j, tg, tg == 0, tg == len(TSETS) - 1, after,
                         tset=ts_)
    run_units()


_NC_CACHE = None


def _get_nc():
    global _NC_CACHE
    if _NC_CACHE is None:
        _NC_CACHE = build_bass()
    return _NC_CACHE


def make_in_maps(queries, keys, values, Wq, Wk, Wv):
    # Host-side shard prep: cast to bf16, transpose activations per batch,
    # slice W column-wise by head group.
    try:
        import ml_dtypes
        bf16 = ml_dtypes.bfloat16
    except ImportError:  # pragma: no cover
        import jax.numpy as jnp
        bf16 = jnp.bfloat16
    xq_t = [np.ascontiguousarray(queries[b].T).astype(bf16) for b in range(B)]
    xk_t = [np.ascontiguousarray(keys[b].T).astype(bf16) for b in range(B)]
    xv_t = [np.ascontiguousarray(values[b].T).astype(bf16) for b in range(B)]
    w_half = [
        (np.ascontiguousarray(Wq[:, g * FPC:(g + 1) * FPC]).astype(bf16),
         np.ascontiguousarray(Wk[:, g * FPC:(g + 1) * FPC]).astype(bf16),
         np.ascontiguousarray(Wv[:, g * FPC:(g + 1) * FPC]).astype(bf16))
        for g in range(2)
    ]

    in_maps = []
    for c in range(N_CORES):
        b, g = c // 2, c % 2
        in_maps.append({
            "xq_t": xq_t[b], "xk_t": xk_t[b], "xv_t": xv_t[b],
            "wq": w_half[g][0], "wk": w_half[g][1], "wv": w_half[g][2],
        })
    return in_maps


def kernel(queries, keys, values, Wq, Wk, Wv, **_):
    queries = np.asarray(queries, dtype=np.float32)
    keys = np.asarray(keys, dtype=np.float32)
    values = np.asarray(values, dtype=np.float32)
    Wq = np.asarray(Wq, dtype=np.float32)
    Wk = np.asarray(Wk, dtype=np.float32)
    Wv = np.asarray(Wv, dtype=np.float32)

    in_maps = make_in_maps(queries, keys, values, Wq, Wk, Wv)

    nc = _get_nc()
    res = run_bass_kernel_spmd(nc, in_maps, list(range(N_CORES)))

    full = np.empty((B, S, H * D), dtype=np.float32)
    for c in range(N_CORES):
        b, g = c // 2, c % 2
        o = res.results[c]["out"]          # [HPC, D+1, S]
        ctx = o[:, :D, :] / o[:, D:D + 1, :]     # [HPC, D, S]
        dst = full[b].reshape(S, H, D)
        dst[:, g * HPC:(g + 1) * HPC, :] = ctx.transpose(2, 0, 1)
    return full
